# revision 1
# baseline (speedup 1.0000x reference)
"""BoxBlur2d (11x11, reflect padding) Trainium2 Bass kernel.

Problem: x [8, 64, 512, 512] f32 -> depthwise 11x11 box blur with reflect
padding on H and W. Separable: apply integer band matrix Mint along H, then
along W, where Mint[i, j] = #taps of output j that read input i (reflection
folded in, values {0,1,2} - exact in fp16); the 1/121 scale is applied in the
final PSUM evacuation.

Sharding: pure data-parallel over the batch dim -> 8 NeuronCores, one batch
image stack [64, 512, 512] per core. x is cast to fp16 on the host (halves
DMA-in traffic; fp16 mantissa rounding ~2^-11 is the only input error since
products by {1,2} and the f32 PSUM accumulation are exact).

Per-core algorithm (all compute on TensorE as fp16 matmuls; both separable
passes fuse a transpose by using the image tile as the stationary operand -
fp16 weights also get the 4x fast-weight-load path):

  pass 1:  u^T[w, h] = sum_{h'} x[h', w] * Mint[h', h]      (u^T = 11*blurH^T)
      for each 128-wide w-chunk: accumulate 4 matmuls over h'-chunks r=0..3
      (lhsT = x[h'-chunk, w-chunk], rhs = Mint[h'-chunk, h-cols]) into one
      PSUM bank [128 w, 512 h]. r=0 streams all 512 h-cols (start=True clears
      the bank's has_written bits); r>=1 stream only the 11-wide band's
      columns (Mint is zero outside).
  pass 2:  y_raw[h, w] = sum_{w'} u^T[w', h] * Mint[w', w]  (y_raw = 121*y)

  PSUM evacuations alternate ScalarE/VectorE; pass-1 casts f32->fp16, pass-2
  scales by 1/121 into f32. DMA out per h-chunk.
"""
import numpy as np
import sys

sys.path.insert(0, "/opt/trn_rl_repo")

import concourse.mybir as mybir
from concourse import bacc
from concourse.tile import TileContext
from concourse import bass_utils

F32 = mybir.dt.float32
F16 = mybir.dt.float16

B, C, H, W = 8, 64, 512, 512
KSIZE = 11
PAD = KSIZE // 2
SCALE = 1.0 / (KSIZE * KSIZE)
NCORES = 8
P = 128
NH = H // P  # 4 contraction chunks

# Streaming column windows for chunks r >= 1 (r == 0 streams all 512 cols
# with start=True): the band cols [128r - PAD, 128r + 128 + PAD), 8B-aligned.
BAND_COLS = {
    1: (122, 262),
    2: (250, 390),
    3: (378, 512),
}


def make_m_matrix() -> np.ndarray:
    """Mint[i, j] = # of taps of output j reading input i (reflect folded)."""
    m = np.zeros((H, H), dtype=np.float64)
    for j in range(H):
        for d in range(-PAD, PAD + 1):
            i = j + d
            if i < 0:
                i = -i
            if i >= H:
                i = 2 * H - 2 - i
            m[i, j] += 1.0
    return m.astype(np.float16)


def build_nc(nch: int = C):
    nc = bacc.Bacc("TRN2", target_bir_lowering=False)
    x_d = nc.dram_tensor("x", [nch, H, W], F16, kind="ExternalInput")
    m_d = nc.dram_tensor("m", [H, H], F16, kind="ExternalInput")
    y_d = nc.dram_tensor("y", [nch, H, W], F32, kind="ExternalOutput")

    with TileContext(nc) as tc:
        with tc.tile_pool(name="const", bufs=1) as cpool, \
             tc.tile_pool(name="xp", bufs=8) as xpool, \
             tc.tile_pool(name="up", bufs=5) as upool, \
             tc.tile_pool(name="yp", bufs=6) as ypool, \
             tc.tile_pool(name="pp", bufs=8, space="PSUM") as ppool:

            # M chunks side by side: m_sb[:, 512r : 512r+512] = M[128r:128(r+1), :]
            m_sb = cpool.tile([P, NH * H], F16)
            for r in range(NH):
                nc.sync.dma_start(m_sb[:, H * r:H * (r + 1)],
                                  m_d[P * r:P * (r + 1), :])

            state = {"evac": 0}

            def emit_pass1(c):
                # x chunks side by side: xt[:, 512r:512r+512] = x[c, 128r:.., :]
                # one packed 3D DMA per channel
                xt = xpool.tile([P, NH * W], F16, tag="x", name=f"xt{c}")
                xs3 = x_d[c].rearrange("(r p) w -> p r w", p=P)
                for k in range(2):
                    nc.sync.dma_start(
                        xt[:, 2 * W * k:2 * W * (k + 1)].rearrange(
                            "p (r w) -> p r w", r=2),
                        xs3[:, 2 * k:2 * (k + 1), :])
                # pass 1: u^T chunks in SBUF, ut[:, 512wc : 512wc+512]
                ut = upool.tile([P, NH * H], F16, tag="u", name=f"ut{c}")
                for wc in range(NH):
                    pu = ppool.tile([P, H], F32, tag="ps", name=f"pu{c}_{wc}")
                    nc.tensor.matmul(pu[:], xt[:, P * wc:P * (wc + 1)],
                                     m_sb[:, 0:H], start=True, stop=False)
                    for r in range(1, NH):
                        c0, c1 = BAND_COLS[r]
                        nc.tensor.matmul(
                            pu[:, c0:c1],
                            xt[:, W * r + P * wc:W * r + P * (wc + 1)],
                            m_sb[:, H * r + c0:H * r + c1],
                            start=False, stop=(r == NH - 1))
                    if state["evac"] % 2 == 0:
                        nc.scalar.copy(ut[:, H * wc:H * (wc + 1)], pu[:])
                    else:
                        nc.vector.tensor_copy(ut[:, H * wc:H * (wc + 1)], pu[:])
                    state["evac"] += 1
                return ut

            def emit_pass2(c, ut):
                # y h-chunks side by side in one tile; one packed out-DMA
                yt = ypool.tile([P, NH * W], F32, tag="y", name=f"yt{c}")
                for hc in range(NH):
                    py = ppool.tile([P, W], F32, tag="ps", name=f"py{c}_{hc}")
                    nc.tensor.matmul(
                        py[:], ut[:, P * hc:P * (hc + 1)],
                        m_sb[:, 0:H], start=True, stop=False)
                    for wc in range(1, NH):
                        c0, c1 = BAND_COLS[wc]
                        nc.tensor.matmul(
                            py[:, c0:c1],
                            ut[:, H * wc + P * hc:H * wc + P * (hc + 1)],
                            m_sb[:, H * wc + c0:H * wc + c1],
                            start=False, stop=(wc == NH - 1))
                    if state["evac"] % 2 == 0:
                        nc.scalar.mul(yt[:, W * hc:W * (hc + 1)], py[:], SCALE)
                    else:
                        nc.vector.tensor_scalar_mul(
                            yt[:, W * hc:W * (hc + 1)], py[:], SCALE)
                    state["evac"] += 1
                # issue out-DMAs from ScalarE (also an HWDGE engine) to
                # split DMA dispatch load across two sequencers
                yd3 = y_d[c].rearrange("(r p) w -> p r w", p=P)
                for k in range(2):
                    nc.scalar.dma_start(
                        yd3[:, 2 * k:2 * (k + 1), :],
                        yt[:, 2 * W * k:2 * W * (k + 1)].rearrange(
                            "p (r w) -> p r w", r=2))

            # software pipeline: emit pass-1 of channel c+1 before pass-2 of
            # channel c so the in-order PE stream has independent matmuls to
            # chew on while channel c's PSUM evacuations drain
            uts = {0: emit_pass1(0)}
            for c in range(nch):
                if c + 1 < nch:
                    uts[c + 1] = emit_pass1(c + 1)
                emit_pass2(c, uts.pop(c))

    nc.compile()
    return nc


_NC_CACHE = None


def _get_nc():
    global _NC_CACHE
    if _NC_CACHE is None:
        _NC_CACHE = build_nc()
    return _NC_CACHE


def to_device_layout(img: np.ndarray) -> np.ndarray:
    """[..., H, W] -> [..., P, NH*W] with [..., p, r*W+w] = [..., 128r+p, w]."""
    lead = img.shape[:-2]
    return np.ascontiguousarray(
        img.reshape(*lead, NH, P, W).swapaxes(-3, -2).reshape(*lead, P, NH * W))


def from_device_layout(dev: np.ndarray) -> np.ndarray:
    lead = dev.shape[:-2]
    return np.ascontiguousarray(
        dev.reshape(*lead, P, NH, W).swapaxes(-3, -2).reshape(*lead, H, W))


def kernel(x: np.ndarray, _run_kwargs: dict | None = None) -> np.ndarray:
    assert x.shape == (B, C, H, W), x.shape
    x16 = np.ascontiguousarray(x.astype(np.float16))
    m = make_m_matrix()
    nc = _get_nc()
    in_maps = [{"x": x16[b], "m": m} for b in range(NCORES)]
    res = bass_utils.run_bass_kernel_spmd(
        nc, in_maps, core_ids=list(range(NCORES)), **(_run_kwargs or {}))
    out = np.stack([res.results[b]["y"] for b in range(NCORES)], axis=0)
    if _run_kwargs:
        kernel.last_results = res
    return out


if __name__ == "__main__":
    # quick CoreSim correctness check on a reduced-channel kernel
    from concourse import bass_interp

    nch = int(sys.argv[1]) if len(sys.argv) > 1 else 4
    rng = np.random.default_rng(0)
    xs = rng.standard_normal((nch, H, W), dtype=np.float32).astype(np.float16)
    nc = build_nc(nch)
    sim = bass_interp.CoreSim(nc)
    sim.tensor("x")[:] = xs
    sim.tensor("m")[:] = make_m_matrix()
    sim.simulate()
    got = np.array(sim.tensor("y"))

    m64 = make_m_matrix().astype(np.float64)
    ref = np.einsum("hj,chw->cjw", m64, xs.astype(np.float64))
    ref = np.einsum("wj,chw->chj", m64, ref) * SCALE
    err = np.abs(got - ref)
    scale = np.abs(ref).max()
    print(f"CoreSim: max_abs={err.max():.3e} rel={err.max() / scale:.3e}")



# revision 3
# speedup vs baseline: 1.3543x; 1.3543x over previous
"""BoxBlur2d (11x11, reflect padding) Trainium2 Bass kernel, v2.

Problem: x [8, 64, 512, 512] f32 -> depthwise 11x11 box blur with reflect
padding on H and W. Separable: y = (1/121) * M^T ... M applied along H then
W, where Mint[i, j] = #taps of output j that read input i (reflection folded
in, values {0,1,2}); band support |i-j| <= 5.

Sharding: pure data-parallel over batch -> 8 NeuronCores, one [64, 512, 512]
image stack per core. Host packs x to fp16 device layout [C, 128, 4*512]
(xdev[c, p, 512r+w] = x[c, 128r+p, w]) so every DMA is 2D-contiguous; y comes
back fp16 in the same layout and is unpacked + upcast on host (fp16 rounding
~2^-11 total error, threshold is 2e-2).

Per-core pipeline (per channel c):
  pass 1: u^T[w, h] = sum_h' x[h', w] * M1[h', h]   (M1 = Mint, fp16)
  pass 2: y[h, w]   = sum_w' u^T[w', h] * M2[w', w] (M2 = Mint/121, fp16;
                      scale folded into M2 so every PSUM evac is a pure copy)

Both passes use the image tile as the stationary operand (fuses the
transpose). Matmuls are BANDED: per 512-col psum bank, each contraction
chunk r streams only its band window [128r-8, 128r+136) (560 cols/bank vs
932 unbanded; measured 299ns vs 485ns per bank). This relies on per-byte
PSUM has_written semantics on hardware: the first matmul (start=True) marks
the whole 2KB bank pending-zero; later matmuls overwrite still-pending cols
and accumulate onto written ones. (CoreSim's uniformity assert rejects this;
run __main__ sim check with sim_safe=True which splits the straddling
matmuls into equivalent uniform pieces.)

Engine budget per core: PE ~155us (8 banded banks/channel), Scalar+Vector
alternate per-pair [128,1024] PSUM evacs (~1.3us each, ~164us/engine), DMA
67MB total ~170us, in-DMAs ganged 4 channels/dispatch on Sync (~150ns each),
out-DMAs ganged on GpSimd. All four walls ~160-180us.
"""
import numpy as np
import sys

sys.path.insert(0, "/opt/trn_rl_repo")

import concourse.mybir as mybir
from concourse import bacc
from concourse.tile import TileContext
from concourse import bass_utils

F32 = mybir.dt.float32
F16 = mybir.dt.float16

B, C, H, W = 8, 64, 512, 512
KSIZE = 11
PAD = KSIZE // 2
SCALE = 1.0 / (KSIZE * KSIZE)
NCORES = 8
P = 128
NH = H // P  # 4 contraction chunks
CW = NH * W  # 2048, per-channel free width in device layout
GRP = 4      # channels per DMA group

# banded col windows per 512-col bank: chunk r covers [128r-8, 128r+136)
BANDS = [(0, 136), (120, 264), (248, 392), (376, 512)]


def make_m_matrix() -> np.ndarray:
    """Mint[i, j] = # of taps of output j reading input i (reflect folded)."""
    m = np.zeros((H, H), dtype=np.float64)
    for j in range(H):
        for d in range(-PAD, PAD + 1):
            i = j + d
            if i < 0:
                i = -i
            if i >= H:
                i = 2 * H - 2 - i
            m[i, j] += 1.0
    return m


def pack_chunks(m: np.ndarray, dtype) -> np.ndarray:
    """[H, H] -> [128, NH*H] with [p, H*r + j] = m[128r + p, j]."""
    return np.ascontiguousarray(
        m.reshape(NH, P, H).transpose(1, 0, 2).reshape(P, NH * H).astype(dtype))


def build_nc(nch: int = C, sim_safe: bool = False):
    nc = bacc.Bacc("TRN2", target_bir_lowering=False)
    x_d = nc.dram_tensor("x", [nch, P, CW], F16, kind="ExternalInput")
    m1_d = nc.dram_tensor("m1", [P, NH * H], F16, kind="ExternalInput")
    m2_d = nc.dram_tensor("m2", [P, NH * H], F16, kind="ExternalInput")
    y_d = nc.dram_tensor("y", [nch, P, CW], F16, kind="ExternalOutput")

    ngrp = (nch + GRP - 1) // GRP
    x3 = x_d.ap().rearrange("c p w -> p c w")
    y3 = y_d.ap().rearrange("c p w -> p c w")

    with TileContext(nc) as tc:
        with tc.tile_pool(name="const", bufs=1) as cpool, \
             tc.tile_pool(name="xg", bufs=3) as xgpool, \
             tc.tile_pool(name="ug", bufs=3) as upool, \
             tc.tile_pool(name="yg", bufs=3) as ygpool, \
             tc.tile_pool(name="pp", bufs=4, space="PSUM") as ppool:

            m1 = cpool.tile([P, NH * H], F16)
            m2 = cpool.tile([P, NH * H], F16)
            nc.sync.dma_start(m1[:], m1_d[:])
            nc.sync.dma_start(m2[:], m2_d[:])

            xg = {}

            def fetch_group(g):
                if g >= ngrp or g in xg:
                    return
                n = min(GRP, nch - GRP * g)
                t = xgpool.tile([P, GRP * CW], F16, tag="xg", name=f"xg{g}")
                nc.sync.dma_start(
                    t[:, 0:n * CW].rearrange("p (c w) -> p c w", c=n),
                    x3[:, GRP * g:GRP * g + n, :])
                xg[g] = t

            state = {"evac": 0}

            def evac(dst_ap, src_ap):
                if state["evac"] % 2 == 0:
                    nc.scalar.copy(dst_ap, src_ap)
                else:
                    nc.vector.tensor_copy(dst_ap, src_ap)
                state["evac"] += 1

            def emit_pass(lhs_tile, lhs_ofs, m_tile, dst_tile, dst_ofs, cname):
                # two psum pairs; each pair = 2 banks (q=0,1); per bank 4
                # banded matmuls; one [128,1024] evac per pair
                for pair in range(2):
                    pt = ppool.tile([P, 2 * H], F32, tag="ps",
                                    name=f"ps_{cname}_{pair}")
                    for q in range(2):
                        bank = 2 * pair + q
                        for r in range(NH):
                            c0, c1 = BANDS[r]
                            if sim_safe and r > 0:
                                # split straddling matmul into uniform pieces
                                cm = BANDS[r - 1][1]
                                nc.tensor.matmul(
                                    pt[:, H * q + c0:H * q + cm],
                                    lhs_tile[:, lhs_ofs + H * r + P * bank:
                                             lhs_ofs + H * r + P * (bank + 1)],
                                    m_tile[:, H * r + c0:H * r + cm],
                                    start=False, stop=False)
                                nc.tensor.matmul(
                                    pt[:, H * q + cm:H * q + c1],
                                    lhs_tile[:, lhs_ofs + H * r + P * bank:
                                             lhs_ofs + H * r + P * (bank + 1)],
                                    m_tile[:, H * r + cm:H * r + c1],
                                    start=False, stop=(r == NH - 1))
                                continue
                            nc.tensor.matmul(
                                pt[:, H * q + c0:H * q + c1],
                                lhs_tile[:, lhs_ofs + H * r + P * bank:
                                         lhs_ofs + H * r + P * (bank + 1)],
                                m_tile[:, H * r + c0:H * r + c1],
                                start=(r == 0), stop=(r == NH - 1))
                    evac(dst_tile[:, dst_ofs + 2 * H * pair:
                                  dst_ofs + 2 * H * (pair + 1)], pt[:])

            def emit_pass1(c):
                g, cig = c // GRP, c % GRP
                u = upool.tile([P, CW], F16, tag="u", name=f"u{c}")
                emit_pass(xg[g], cig * CW, m1, u, 0, f"p1c{c}")
                return u

            yg = {}

            def emit_pass2(c, u):
                g, cig = c // GRP, c % GRP
                if cig == 0:
                    yg[g] = ygpool.tile([P, GRP * CW], F16, tag="yg",
                                        name=f"yg{g}")
                emit_pass(u, 0, m2, yg[g], cig * CW, f"p2c{c}")
                if cig == GRP - 1 or c == nch - 1:
                    n = min(GRP, nch - GRP * g)
                    nc.gpsimd.dma_start(
                        y3[:, GRP * g:GRP * g + n, :],
                        yg[g][:, 0:n * CW].rearrange("p (c w) -> p c w", c=n))
                    del yg[g]

            fetch_group(0)
            fetch_group(1)
            us = {0: emit_pass1(0)}
            for c in range(nch):
                if c % GRP == 0:
                    fetch_group(c // GRP + 2)
                if c + 1 < nch:
                    us[c + 1] = emit_pass1(c + 1)
                emit_pass2(c, us.pop(c))

    nc.compile()
    return nc


_NC_CACHE = None


def _get_nc():
    global _NC_CACHE
    if _NC_CACHE is None:
        _NC_CACHE = build_nc()
    return _NC_CACHE


def to_device_layout(img: np.ndarray) -> np.ndarray:
    """[..., H, W] -> [..., P, NH*W] with [..., p, r*W+w] = [..., 128r+p, w]."""
    lead = img.shape[:-2]
    return np.ascontiguousarray(
        img.reshape(*lead, NH, P, W).swapaxes(-3, -2).reshape(*lead, P, NH * W))


def from_device_layout(dev: np.ndarray) -> np.ndarray:
    lead = dev.shape[:-2]
    return np.ascontiguousarray(
        dev.reshape(*lead, P, NH, W).swapaxes(-3, -2).reshape(*lead, H, W))


def kernel(x: np.ndarray, _run_kwargs: dict | None = None) -> np.ndarray:
    assert x.shape == (B, C, H, W), x.shape
    xdev = to_device_layout(x.astype(np.float16))
    mint = make_m_matrix()
    m1 = pack_chunks(mint, np.float16)
    m2 = pack_chunks(mint * SCALE, np.float16)
    nc = _get_nc()
    in_maps = [{"x": xdev[b], "m1": m1, "m2": m2} for b in range(NCORES)]
    res = bass_utils.run_bass_kernel_spmd(
        nc, in_maps, core_ids=list(range(NCORES)), **(_run_kwargs or {}))
    ydev = np.stack([res.results[b]["y"] for b in range(NCORES)], axis=0)
    out = from_device_layout(ydev).astype(np.float32)
    if _run_kwargs:
        kernel.last_results = res
    return out


if __name__ == "__main__":
    # CoreSim correctness check on a reduced-channel kernel (sim_safe split)
    from concourse import bass_interp

    nch = int(sys.argv[1]) if len(sys.argv) > 1 else 4
    rng = np.random.default_rng(0)
    xs = rng.standard_normal((nch, H, W), dtype=np.float32).astype(np.float16)
    nc = build_nc(nch, sim_safe=True)
    sim = bass_interp.CoreSim(nc)
    sim.tensor("x")[:] = to_device_layout(xs)
    mint = make_m_matrix()
    sim.tensor("m1")[:] = pack_chunks(mint, np.float16)
    sim.tensor("m2")[:] = pack_chunks(mint * SCALE, np.float16)
    sim.simulate()
    got = from_device_layout(np.array(sim.tensor("y"))).astype(np.float64)

    ref = np.einsum("hj,chw->cjw", mint, xs.astype(np.float64))
    ref = np.einsum("wj,chw->chj", mint, ref) * SCALE
    err = np.abs(got - ref)
    scale = np.abs(ref).max()
    print(f"CoreSim: max_abs={err.max():.3e} rel={err.max() / scale:.3e}")
